# revision 32
# baseline (speedup 1.0000x reference)
"""Trainium2 Bass kernel for nn_AttentionModeEncoder (B=4, S=2048, HID=1024, 16 heads x 64).

Sharding: 8 cores = 4 batches x 2 head-groups (8 heads / 512 features per core).

Host side: casts x/weights to bf16, gathers the unmasked key positions
(mask==1, ~1024 of 2048) per batch into a padded xg[M=1280] buffer, and
passes an additive mask-bias vector (0 real / -1e9 pad).  Attention over
gathered keys is mathematically identical to the reference's masked
softmax (masked logits clamp to -50 -> weight ~e-22).

Per core (batch b, head-group g):
  Phase A: x^T, xg^T, Wq^T/Wk^T/Wv^T/Wo^T built by DMA-XBAR transposes
    straight from DRAM (bf16).  Q^T = Wq @ x^T (all 2048 q), K^T = Wk @ xg^T
    (M keys), V = xg @ Wv^T computed directly in [k, d] layout by using xg^T
    tiles as the stationary operand (bias via a ones-row matmul).  All
    matmuls bf16 (1 cycle/row vs 4 for fp32).  Bias adds on ScalarE.
  Phase B: per (head, 1024-wide q chunk): S^T[k,q] = K^T.T @ Q^T row-packed
    in 64-contraction pairs, P = exp(0.125*S + maskbias) on ScalarE (bf16),
    AV with a ones-augmented V column giving softmax denominators for free,
    fp32r PE broadcast + fast reciprocal + DVE multiply to normalize.
  Phase C: partial out-projection y^T = Wo[:, cslice] @ attn^T with bias on
    ScalarE, streamed to DRAM.
Host sums the two partials per batch (cross-head-group reduction).
"""

import os
import sys
import numpy as np
from contextlib import ExitStack

for _p in ("/opt/trn_rl_repo", "/root/.axon_site/_ro/trn_rl_repo"):
    if os.path.isdir(_p) and _p not in sys.path:
        sys.path.insert(0, _p)

import ml_dtypes
import concourse.bass as bass
import concourse.bacc as bacc
import concourse.mybir as mybir
import concourse.tile as tile

B, S, HID = 4, 2048, 1024
JC = 512                 # features per core (8 heads)
M_GATHER = 1280          # padded gathered-key count (10 k-tiles)
NCORES = 8
FP = mybir.dt.float32
FR = mybir.dt.float32r
BF = mybir.dt.bfloat16
MULT = mybir.AluOpType.mult
EXP = mybir.ActivationFunctionType.Exp
IDENT = mybir.ActivationFunctionType.Identity
COPY = mybir.ActivationFunctionType.Copy
BF_NP = ml_dtypes.bfloat16

TRACE = False
LAST_RESULTS = {}


def build_nc(M=M_GATHER, debug=False):
    NKT = M // 128           # k-tiles
    nc = bacc.Bacc()
    dbg = {}
    if debug:
        dbg["xT"] = nc.declare_dram_parameter("dbg_xT", [128, 8, S], BF, isOutput=True)
        dbg["wqT"] = nc.declare_dram_parameter("dbg_wqT", [128, 8, JC], BF, isOutput=True)
        dbg["QTd"] = nc.declare_dram_parameter("dbg_QTd", [128, 8, S], BF, isOutput=True)
        dbg["KTd"] = nc.declare_dram_parameter("dbg_KTd", [128, 8, M], BF, isOutput=True)
        dbg["vaug"] = nc.declare_dram_parameter("dbg_vaug", [128, NKT, 8, 128], BF, isOutput=True)
        dbg["PT"] = nc.declare_dram_parameter("dbg_PT", [128, NKT, 1024], BF, isOutput=True)
        dbg["avp"] = nc.declare_dram_parameter("dbg_avp", [128, 1024], FP, isOutput=True)
        dbg["recb"] = nc.declare_dram_parameter("dbg_recb", [64, 1024], FP, isOutput=True)
        dbg["outT"] = nc.declare_dram_parameter("dbg_outT", [128, 4, S], BF, isOutput=True)
    x = nc.declare_dram_parameter("x", [S, HID], BF, isOutput=False)
    xg = nc.declare_dram_parameter("xg", [M, HID], BF, isOutput=False)
    mbias = nc.declare_dram_parameter("mbias", [M], FP, isOutput=False)
    wq = nc.declare_dram_parameter("wq", [JC, HID], BF, isOutput=False)
    bq = nc.declare_dram_parameter("bq", [JC], FP, isOutput=False)
    wk = nc.declare_dram_parameter("wk", [JC, HID], BF, isOutput=False)
    bk = nc.declare_dram_parameter("bk", [JC], FP, isOutput=False)
    wv = nc.declare_dram_parameter("wv", [JC, HID], BF, isOutput=False)
    bv = nc.declare_dram_parameter("bv", [JC], BF, isOutput=False)
    wo = nc.declare_dram_parameter("wo", [HID, JC], BF, isOutput=False)
    bo = nc.declare_dram_parameter("bo", [HID], FP, isOutput=False)
    y = nc.declare_dram_parameter("y", [HID, S], FP, isOutput=True)

    # K-proj column chunks covering M (moving dim <= 512)
    kchunks = []
    c0 = 0
    while c0 < M:
        cw = min(512, M - c0)
        kchunks.append((c0, cw))
        c0 += cw

    with tile.TileContext(nc) as tc, ExitStack() as ctx:
        const = ctx.enter_context(tc.tile_pool(name="const", bufs=1))
        mid = ctx.enter_context(tc.tile_pool(name="mid", bufs=1))

        onescol = const.tile([1, 128], BF)
        nc.vector.memset(onescol[:], 1.0)

        # warm the ScalarE exp spline table during phase A so the first
        # real exp in phase B doesn't pay the ACT_TABLE_LOAD
        warm = const.tile([1, 1], FP, tag="warm")
        nc.vector.memset(warm[:], 0.0)
        nc.scalar.activation(warm[:], warm[:], EXP)

        maskA = const.tile([128, NKT], FP)
        nc.sync.dma_start(out=maskA[:], in_=mbias.rearrange("(kt p) -> p kt", p=128))

        bqt = const.tile([128, 4], FP, tag="bqt")
        nc.sync.dma_start(out=bqt[:], in_=bq.rearrange("(o p) -> p o", p=128))
        bkt = const.tile([128, 4], FP, tag="bkt")
        nc.sync.dma_start(out=bkt[:], in_=bk.rearrange("(o p) -> p o", p=128))
        bvr = const.tile([1, JC], BF, tag="bvr")
        nc.sync.dma_start(out=bvr[:], in_=bv.rearrange("(a j) -> a j", a=1))
        bot = const.tile([128, 8], FP, tag="bot")
        nc.sync.dma_start(out=bot[:], in_=bo.rearrange("(o p) -> p o", p=128))

        # persistent tensors.  QTd/KTd hold each head's 64 feature rows
        # DUPLICATED into both partition halves so k-tile pairs can be
        # row-packed into both halves of the PE array concurrently.
        KTd = mid.tile([128, 8, M], BF)          # [dup-half x d, head, k]
        QTd = mid.tile([128, 8, S], BF)
        # V aug: [k, kt, head, d|ones].  Columns 64:128 are all-ones so the
        # AV matmul replicates the softmax denominator into PSUM partitions
        # 64:127 -- no separate broadcast needed for the normalize.
        vaug = mid.tile([128, NKT, 8, 128], BF)
        nc.vector.memset(vaug[:, :, :, 64:128], 1.0)
        outT = mid.tile([128, 4, S], BF)         # attention out^T (c-major)
        woT = mid.tile([128, 4, HID], BF)        # [c-part, ct, o]

        # ------------- Phase A: DMA transposes + Q/K/V projections -------------
        with ExitStack() as actx:
            tpool = actx.enter_context(tc.tile_pool(name="tpool", bufs=1))
            pps = actx.enter_context(tc.tile_pool(name="pps", bufs=6, space="PSUM"))

            wkT = tpool.tile([128, 8, JC], BF, tag="wkT")
            wvT = tpool.tile([128, 8, JC], BF, tag="wvT")
            wqT = tpool.tile([128, 8, JC], BF, tag="wqT")
            xgT = tpool.tile([128, 8, M], BF, tag="xgT")
            xT = tpool.tile([128, 8, S], BF, tag="xT")

            # All XBAR transposes on the SP queue: concurrent DMA transposes
            # from two HWDGE queues corrupt each other on real HW, and one
            # whole-tensor transpose serializes on a single DMA engine.
            # Per-slice transposes in consumer-chase order: K-projection
            # inputs (wk/xg interleaved) first so PE can start ~4us in, then
            # wv (V proj), wq, and xT slice-by-slice for the Q projection.
            for it in range(8):
                nc.sync.dma_start_transpose(
                    wkT[:, it, :], wk[:, it * 128:(it + 1) * 128])
                nc.sync.dma_start_transpose(
                    xgT[:, it, :], xg[:, it * 128:(it + 1) * 128])
            for it in range(8):
                nc.sync.dma_start_transpose(
                    wvT[:, it, :], wv[:, it * 128:(it + 1) * 128])
            for it in range(8):
                nc.sync.dma_start_transpose(
                    wqT[:, it, :], wq[:, it * 128:(it + 1) * 128])
            for it in range(8):
                nc.sync.dma_start_transpose(
                    xT[:, it, :], x[:, it * 128:(it + 1) * 128])
            for ct in range(4):
                nc.sync.dma_start_transpose(
                    woT[:, ct, :], wo[:, ct * 128:(ct + 1) * 128])

            # K projection: K^T[j, k] for gathered keys
            for jt in range(4):
                psums = [
                    pps.tile([128, 512], FP, tag="pp", name=f"ppk{i}")
                    for i in range(len(kchunks))
                ]
                for it in range(8):
                    for ci, (c0, cw) in enumerate(kchunks):
                        nc.tensor.matmul(
                            psums[ci][:, 0:cw],
                            lhsT=wkT[:, it, jt * 128:(jt + 1) * 128],
                            rhs=xgT[:, it, c0:c0 + cw],
                            start=(it == 0), stop=(it == 7),
                        )
                for ci, (c0, cw) in enumerate(kchunks):
                    for hh in range(2):
                        p0 = hh * 64
                        nc.scalar.activation(
                            KTd[p0:p0 + 64, jt * 2 + hh, c0:c0 + cw],
                            psums[ci][p0:p0 + 64, 0:cw],
                            IDENT, bias=bkt[p0:p0 + 64, jt:jt + 1],
                        )
            # duplicate each head's rows into the opposite partition half
            for h in range(8):
                src = h % 2 * 64
                dst = 64 - src
                nc.sync.dma_start(
                    out=KTd[dst:dst + 64, h, :], in_=KTd[src:src + 64, h, :])

            # V projection, transposed: V[k, j] via stationary xg^T tiles,
            # bias via ones-row matmul.
            for tt in range(NKT):
                vp = pps.tile([128, 512], FP, tag="pp", name="ppv")
                for it in range(8):
                    nc.tensor.matmul(
                        vp[:],
                        lhsT=xgT[:, it, tt * 128:(tt + 1) * 128],
                        rhs=wvT[:, it, :],
                        start=(it == 0), stop=(it == 7),
                    )
                nc.tensor.matmul(
                    vp[:], lhsT=onescol[:], rhs=bvr[:],
                    start=False, stop=True, skip_group_check=True,
                )
                nc.scalar.activation(vaug[:, tt, :, 0:64], vp[:], COPY)

            # Q projection: Q^T[j, t] over all 2048 queries
            for jt in range(4):
                psums = [
                    pps.tile([128, 512], FP, tag="pp", name=f"ppq{i}")
                    for i in range(4)
                ]
                for it in range(8):
                    for tq in range(4):
                        nc.tensor.matmul(
                            psums[tq][:],
                            lhsT=wqT[:, it, jt * 128:(jt + 1) * 128],
                            rhs=xT[:, it, tq * 512:(tq + 1) * 512],
                            start=(it == 0), stop=(it == 7),
                        )
                for tq in range(4):
                    t0 = tq * 512
                    for hh in range(2):
                        p0 = hh * 64
                        nc.scalar.activation(
                            QTd[p0:p0 + 64, jt * 2 + hh, t0:t0 + 512],
                            psums[tq][p0:p0 + 64, :],
                            IDENT, bias=bqt[p0:p0 + 64, jt:jt + 1],
                        )
                # dup this jt's heads immediately so phase B's first blocks
                # aren't gated on the last jt's adds
                for hh in range(2):
                    h = jt * 2 + hh
                    src = hh * 64
                    dst = 64 - src
                    nc.sync.dma_start(
                        out=QTd[dst:dst + 64, h, :], in_=QTd[src:src + 64, h, :])

            if debug:
                nc.sync.dma_start(out=dbg["xT"][:], in_=xT[:])
                nc.sync.dma_start(out=dbg["wqT"][:], in_=wqT[:])
                nc.sync.dma_start(out=dbg["QTd"][:], in_=QTd[:])
                nc.sync.dma_start(out=dbg["KTd"][:], in_=KTd[:])
                nc.sync.dma_start(out=dbg["vaug"][:], in_=vaug[:])

        # ------------- Phase B + C: attention with interleaved out-proj ----
        with ExitStack() as bctx:
            ptpool = bctx.enter_context(tc.tile_pool(name="ptpool", bufs=3))
            rpool = bctx.enter_context(tc.tile_pool(name="rpool", bufs=3))
            ypool = bctx.enter_context(tc.tile_pool(name="ypool", bufs=2))
            spool = bctx.enter_context(tc.tile_pool(name="spool", bufs=2, space="PSUM"))
            avpool = bctx.enter_context(
                tc.tile_pool(name="avpool", bufs=1, space="PSUM"))
            ypsum = bctx.enter_context(
                tc.tile_pool(name="ypsum", bufs=1, space="PSUM"))

            def outproj_chain(ot, tc_i):
                """One 128-row out-projection chain for t chunk tc_i."""
                yps = ypsum.tile([128, 1024], FP, tag="yps")
                for ct in range(4):
                    for qq in range(2):
                        nc.tensor.matmul(
                            yps[:, qq * 512:(qq + 1) * 512],
                            lhsT=woT[:, ct, ot * 128:(ot + 1) * 128],
                            rhs=outT[:, ct,
                                     tc_i * 1024 + qq * 512:
                                     tc_i * 1024 + (qq + 1) * 512],
                            start=(ct == 0), stop=(ct == 3),
                        )
                yt = ypool.tile([128, 1024], FP, tag="yt")
                nc.scalar.activation(
                    yt[:], yps[:], IDENT, bias=bot[:, ot:ot + 1])
                nc.sync.dma_start(
                    out=y[ot * 128:(ot + 1) * 128,
                          tc_i * 1024:(tc_i + 1) * 1024],
                    in_=yt[:],
                )

            for qc in range(2):                  # q chunks of 1024
                q0 = qc * 1024
                for h in range(8):
                    avp = avpool.tile([128, 1024], FP, tag="av")
                    PTt = ptpool.tile([128, NKT, 1024], BF, tag="PT")
                    for pr in range((NKT + 1) // 2):   # k-tile pairs, row-packed
                        k0 = pr * 2
                        sp_a = spool.tile([128, 1024], FP, tag="sp", name="sp_a")
                        sp_b = None
                        if k0 + 1 < NKT:
                            sp_b = spool.tile([128, 1024], FP, tag="sp", name="sp_b")
                        # same-stationary matmuls back-to-back pipeline best
                        # on HW (measured: grouped beats a/b alternation)
                        for qq in range(2):
                            qs = slice(q0 + qq * 512, q0 + (qq + 1) * 512)
                            ps = slice(qq * 512, (qq + 1) * 512)
                            nc.tensor.matmul(
                                sp_a[:, ps],
                                lhsT=KTd[0:64, h, k0 * 128:(k0 + 1) * 128],
                                rhs=QTd[0:64, h, qs],
                                start=True, stop=True,
                            )
                        if sp_b is not None:
                            for qq in range(2):
                                qs = slice(q0 + qq * 512, q0 + (qq + 1) * 512)
                                ps = slice(qq * 512, (qq + 1) * 512)
                                nc.tensor.matmul(
                                    sp_b[:, ps],
                                    lhsT=KTd[64:128, h, (k0 + 1) * 128:(k0 + 2) * 128],
                                    rhs=QTd[64:128, h, qs],
                                    start=True, stop=True,
                                )
                        nc.scalar.activation(
                            PTt[:, k0, :], sp_a[:], EXP,
                            bias=maskA[:, k0:k0 + 1], scale=0.125,
                        )
                        if sp_b is not None:
                            nc.scalar.activation(
                                PTt[:, k0 + 1, :], sp_b[:], EXP,
                                bias=maskA[:, k0 + 1:k0 + 2], scale=0.125,
                            )
                    for kt in range(NKT):
                        for qq in range(2):
                            nc.tensor.matmul(
                                avp[:, qq * 512:(qq + 1) * 512],
                                lhsT=vaug[:, kt, h, :],
                                rhs=PTt[:, kt, qq * 512:(qq + 1) * 512],
                                start=(kt == 0), stop=(kt == NKT - 1),
                                skip_group_check=True,
                            )
                    # normalize: avp[64:128] holds the denominator per q.
                    # Shifted copy down to base partition 0: custom-DVE ops
                    # (reciprocal) and tensor_tensor need base-0 / aligned
                    # operands on real HW.
                    den = rpool.tile([64, 1024], FP, tag="den")
                    nc.vector.tensor_copy(out=den[:], in_=avp[64:128, :])
                    recb = rpool.tile([64, 1024], FP, tag="recb")
                    nc.vector.reciprocal_approx_fast(recb[:], den[:])
                    if debug and h == 0 and qc == 0:
                        nc.sync.dma_start(out=dbg["PT"][:], in_=PTt[:])
                        avs = rpool.tile([128, 1024], FP, tag="avs")
                        nc.vector.tensor_copy(out=avs[:], in_=avp[:])
                        nc.sync.dma_start(out=dbg["avp"][:], in_=avs[:])
                        nc.sync.dma_start(out=dbg["recb"][:], in_=recb[:])
                    nc.vector.tensor_tensor(
                        outT[(h % 2) * 64:(h % 2) * 64 + 64, h // 2, q0:q0 + 1024],
                        avp[0:64, :], recb[:], MULT,
                    )
                    if qc == 1:
                        # t chunk 0 of the out-projection, one 128-row chain
                        # per head block: fills PE gaps while ScalarE exps.
                        outproj_chain(h, 0)

            if debug:
                nc.sync.dma_start(out=dbg["outT"][:], in_=outT[:])

            for ot in range(8):                  # t chunk 1 tail
                outproj_chain(ot, 1)
    return nc


_NCS = {}


def _get_nc(M=M_GATHER):
    if M not in _NCS:
        ncc = build_nc(M)
        ncc.finalize()   # run Bacc passes (reg alloc, wait splitting)
        _NCS[M] = ncc
    return _NCS[M]


def make_in_maps(x, mask, Wq, bq, Wk, bk, Wv, bv, Wo, bo, M=M_GATHER):
    bf = lambda a: np.ascontiguousarray(np.asarray(a, np.float32).astype(BF_NP))
    f32 = lambda a: np.ascontiguousarray(np.asarray(a, dtype=np.float32))
    in_maps = []
    for c in range(NCORES):
        b, g = c // 2, c % 2
        sl = slice(g * JC, (g + 1) * JC)
        xb = np.asarray(x[b], np.float32)
        mb = np.asarray(mask[b])
        idx = np.nonzero(mb != 0)[0]
        m = len(idx)
        assert m <= M, (m, M)
        xgb = np.zeros((M, HID), np.float32)
        xgb[:m] = xb[idx]
        mbias = np.full(M, -1e9, np.float32)
        mbias[:m] = 0.0
        in_maps.append({
            "x": bf(xb), "xg": bf(xgb), "mbias": mbias,
            "wq": bf(Wq[sl]), "bq": f32(bq[sl]),
            "wk": bf(Wk[sl]), "bk": f32(bk[sl]),
            "wv": bf(Wv[sl]), "bv": bf(bv[sl]),
            "wo": bf(Wo[:, sl]),
            "bo": f32(bo) if g == 0 else np.zeros(HID, np.float32),
        })
    return in_maps


def kernel(x, mask, Wq, bq, Wk, bk, Wv, bv, Wo, bo):
    from concourse.bass_utils import run_bass_kernel_spmd

    maxm = int(max((np.asarray(mask[b]) != 0).sum() for b in range(B)))
    M = min(max(2, -(-maxm // 128)) * 128, S)
    nc = _get_nc(M)
    in_maps = make_in_maps(x, mask, Wq, bq, Wk, bk, Wv, bv, Wo, bo, M=M)
    kw = {}
    if TRACE:
        import shutil
        shutil.rmtree("/root/problem/trace_out", ignore_errors=True)
        os.makedirs("/root/problem/trace_out", exist_ok=True)
        kw = dict(tmpdir="/root/problem/trace_out")
    r = run_bass_kernel_spmd(nc, in_maps, list(range(NCORES)), trace=TRACE, **kw)
    LAST_RESULTS["exec_time_ns"] = r.exec_time_ns
    LAST_RESULTS["mean_exec_time_ns"] = r.mean_exec_time_ns
    y = np.empty((B, S, HID), np.float32)
    for b in range(B):
        y[b] = (r.results[2 * b]["y"] + r.results[2 * b + 1]["y"]).T
    return y
